# revision 5
# baseline (speedup 1.0000x reference)
"""Distributed multi-head attention kernel for one TRN2 chip (8 NeuronCores).

Problem (hardcoded): x[2,2048,1024], Wq/Wk/Wv[1024,512], Wout[512,1024], bout[1024]
  H=8 heads, HD=64. reference: softmax((xWq)(xWk)^T/sqrt(64)) (xWv) @ Wout + bout

Sharding: one head per core (tensor parallel). Each core gets the full x
(pre-transposed on host to x.T [1024, 4096] so projections contract on the
partition axis), its head's 64-column slices of Wq/Wk/Wv, and the full Wout.
After per-head attention, an AllToAll reshards from head-parallel to
sequence-parallel (each core ends up with all 8 heads for its 512 tokens),
then each core computes its [512, 1024] slice of the output projection.
Host concatenates the 8 slices.

Layout notes:
 - projections computed transposed (q.T/k.T/v.T [64, N]) with W-chunks
   stationary; v is PE-transposed back to row-major [N, 64] and augmented
   with a ones column so attn@v also yields the softmax denominator Z.
 - scores computed transposed (s.T [j, i]) so exp(s).T feeds attn@v directly;
   softmax max-subtraction is skipped (scores are O(1), exp can't overflow).
 - 1/Z broadcast across partitions via a K=1 matmul with a ones vector.
 - attention scale 1/8 is folded into Wq at load time.
"""

import os
import numpy as np

import concourse.bass as bass
import concourse.mybir as mybir
import concourse.tile as tile
from concourse import bacc
from concourse.bass_utils import run_bass_kernel_spmd
from concourse.masks import make_identity

F32 = mybir.dt.float32
BF16 = mybir.dt.bfloat16

B, N, QDIM = 2, 2048, 1024
H, HD = 8, 64
INNER = H * HD  # 512
NCORES = 8
BN = B * N        # 4096
NT = N            # tokens per batch
SHARD = BN // NCORES  # 512 rows per core of the final output
P = 128
KC = QDIM // P    # 8 feature chunks of x
JC = NT // P      # 16 key chunks per batch
FB = 512          # free-dim block (psum bank width in f32)
IB = NT // FB     # 4 query blocks per batch

_NC = None  # cached compiled graph
LAST_RESULTS = None  # BassKernelResults of the most recent run


def build_nc():
    nc = bacc.Bacc(None, target_bir_lowering=False, num_devices=NCORES)

    xT = nc.declare_dram_parameter("xT", [QDIM, BN], F32, isOutput=False)
    wq = nc.declare_dram_parameter("wq", [QDIM, HD], F32, isOutput=False)
    wk = nc.declare_dram_parameter("wk", [QDIM, HD], F32, isOutput=False)
    wv = nc.declare_dram_parameter("wv", [QDIM, HD], F32, isOutput=False)
    wout = nc.declare_dram_parameter("wout", [INNER, QDIM], F32, isOutput=False)
    bout = nc.declare_dram_parameter("bout", [1, QDIM], F32, isOutput=False)
    out = nc.declare_dram_parameter("out", [SHARD, QDIM], F32, isOutput=True)

    with tile.TileContext(nc) as tc:
        with (
            tc.tile_pool(name="consts", bufs=1) as constp,
            tc.tile_pool(name="stage", bufs=2) as stagep,
            tc.tile_pool(name="xbp", bufs=8) as xbp,
            tc.tile_pool(name="qkv", bufs=2) as qkvp,
            tc.tile_pool(name="vres", bufs=32) as vresp,
            tc.tile_pool(name="et", bufs=3) as etp,
            tc.tile_pool(name="un", bufs=3) as unp,
            tc.tile_pool(name="outs", bufs=2) as outp,
            tc.tile_pool(name="atp", bufs=4) as atp,
            tc.tile_pool(name="dram", bufs=1, space="DRAM") as dramp,
            tc.tile_pool(name="ps_sc", bufs=2, space="PSUM") as ps_sc,
            tc.tile_pool(name="ps_acc", bufs=2, space="PSUM") as ps_acc,
            tc.tile_pool(name="ps_misc", bufs=2, space="PSUM") as ps_misc,
        ):
            # === constants ===
            ident = constp.tile([P, P], BF16, tag="ident")
            make_identity(nc, ident)
            ones1 = constp.tile([1, HD], BF16, tag="ones1")
            nc.gpsimd.memset(ones1, 1.0)
            ones128 = constp.tile([1, P], F32, tag="ones128")
            nc.gpsimd.memset(ones128, 1.0)

            # bias broadcast across partitions: K=1 f32 matmul with ones
            bias_row = constp.tile([1, QDIM], F32, tag="bias_row")
            nc.sync.dma_start(bias_row, bout[0:1, :])
            bias_sb = constp.tile([P, QDIM], F32, tag="bias_sb")
            for fh in range(2):
                pb = ps_misc.tile([P, FB], F32, tag="ps_misc")
                nc.tensor.matmul(
                    pb, ones128, bias_row[:, fh * FB:(fh + 1) * FB],
                    start=True, stop=True,
                )
                nc.vector.tensor_copy(bias_sb[:, fh * FB:(fh + 1) * FB], pb)

            # === load x.T, cast to bf16 ===
            xb = []
            for c in range(KC):
                xs = stagep.tile([P, BN], F32, tag="xstage")
                nc.sync.dma_start(xs, xT[c * P:(c + 1) * P, :])
                t = xbp.tile([P, BN], BF16, tag="xb")
                nc.vector.tensor_copy(t, xs)
                xb.append(t)

            # === load weights, cast to bf16 (fold 1/sqrt(HD) into Wq) ===
            wqb, wkb, wvb = [], [], []
            for (w_ext, lst, scale, nm) in (
                (wq, wqb, HD ** -0.5, "wq"),
                (wk, wkb, None, "wk"),
                (wv, wvb, None, "wv"),
            ):
                for c in range(KC):
                    ws = stagep.tile([P, HD], F32, tag="wstage")
                    nc.sync.dma_start(ws, w_ext[c * P:(c + 1) * P, :])
                    t = constp.tile([P, HD], BF16, tag=f"{nm}{c}")
                    if scale is not None:
                        nc.vector.tensor_scalar_mul(t, ws, scale)
                    else:
                        nc.vector.tensor_copy(t, ws)
                    lst.append(t)
            woutb = []
            for c in range(INNER // P):
                ws = stagep.tile([P, QDIM], F32, tag="wostage")
                nc.sync.dma_start(ws, wout[c * P:(c + 1) * P, :])
                t = constp.tile([P, QDIM], BF16, tag=f"wo{c}")
                nc.vector.tensor_copy(t, ws)
                woutb.append(t)

            # A2A buffers: [8 shards, 64, 512] == [512, 512]
            a2a_in = dramp.tile([NCORES * HD, SHARD], BF16, tag="a2a_in")
            a2a_out = dramp.tile([NCORES * HD, SHARD], BF16, tag="a2a_out")

            # === per-batch attention ===
            for b in range(B):
                qT = qkvp.tile([HD, NT], BF16, tag="qT")
                kT = qkvp.tile([HD, NT], BF16, tag="kT")
                vT = qkvp.tile([HD, NT], BF16, tag="vT")
                for (wchunks, dst) in ((wqb, qT), (wkb, kT), (wvb, vT)):
                    for ib in range(IB):
                        ps = ps_misc.tile([HD, FB], F32, tag="ps_misc")
                        for c in range(KC):
                            nc.tensor.matmul(
                                ps, wchunks[c],
                                xb[c][:, b * NT + ib * FB: b * NT + (ib + 1) * FB],
                                start=(c == 0), stop=(c == KC - 1),
                            )
                        nc.vector.tensor_copy(dst[:, ib * FB:(ib + 1) * FB], ps)

                # v back to row-major, augmented with a ones column
                vsb = []
                for jc in range(JC):
                    pt = ps_misc.tile([P, HD], BF16, tag="ps_misc")
                    nc.tensor.transpose(
                        pt, vT[:, jc * P:(jc + 1) * P], ident[0:HD, 0:HD]
                    )
                    vt = vresp.tile([P, HD + 8], BF16, tag="vsb")
                    nc.vector.tensor_copy(vt[:, 0:HD], pt)
                    nc.gpsimd.memset(vt[:, HD:HD + 1], 1.0)
                    vsb.append(vt)

                # attention: scores.T -> exp -> (e.T)@v_aug, in two query halves
                for half in range(2):
                    accs = [
                        ps_acc.tile([HD + 1, FB], F32, tag="ps_acc",
                                    name=f"acc{b}_{half}_{q2}")
                        for q2 in range(2)
                    ]
                    for jc in range(JC):
                        ss = ps_sc.tile([P, 2 * FB], F32, tag="ps_sc")
                        for q2 in range(2):
                            i0 = half * 2 * FB + q2 * FB
                            nc.tensor.matmul(
                                ss[:, q2 * FB:(q2 + 1) * FB],
                                kT[:, jc * P:(jc + 1) * P],
                                qT[:, i0:i0 + FB],
                                start=True, stop=True,
                            )
                        et = etp.tile([P, 2 * FB], BF16, tag="et")
                        nc.scalar.activation(
                            et, ss, mybir.ActivationFunctionType.Exp
                        )
                        for q2 in range(2):
                            nc.tensor.matmul(
                                accs[q2], vsb[jc][:, 0:HD + 1],
                                et[:, q2 * FB:(q2 + 1) * FB],
                                start=(jc == 0), stop=(jc == JC - 1),
                            )
                    for q2 in range(2):
                        ib = half * 2 + q2
                        rz = unp.tile([1, FB], BF16, tag="rz")
                        with nc.allow_low_precision("bf16 softmax denom ok"):
                            nc.vector.reciprocal(rz, accs[q2][HD:HD + 1, :])
                        pb = ps_misc.tile([HD, FB], F32, tag="ps_misc")
                        nc.tensor.matmul(pb, ones1, rz, start=True, stop=True)
                        rb = unp.tile([HD, FB], BF16, tag="rb")
                        nc.vector.tensor_copy(rb, pb)
                        un = unp.tile([HD, FB], BF16, tag="un")
                        nc.vector.tensor_mul(un, accs[q2][0:HD, :], rb)
                        s = b * IB + ib  # destination shard = global row block
                        nc.sync.dma_start(a2a_in[s * HD:(s + 1) * HD, :], un)

            # === reshard: head-parallel -> sequence-parallel ===
            nc.gpsimd.collective_compute(
                "AllToAll",
                mybir.AluOpType.bypass,
                replica_groups=[list(range(NCORES))],
                ins=[a2a_in.opt()],
                outs=[a2a_out.opt()],
            )

            # === output projection on my 512 rows ===
            at = []
            for c in range(INNER // P):
                t = atp.tile([P, SHARD], BF16, tag=f"at{c}")
                nc.sync.dma_start(t, a2a_out[c * P:(c + 1) * P, :])
                at.append(t)
            for ic in range(SHARD // P):
                ot = outp.tile([P, QDIM], F32, tag="ot")
                for fh in range(2):
                    po = ps_misc.tile([P, FB], F32, tag="ps_misc")
                    for c in range(INNER // P):
                        nc.tensor.matmul(
                            po, at[c][:, ic * P:(ic + 1) * P],
                            woutb[c][:, fh * FB:(fh + 1) * FB],
                            start=(c == 0), stop=(c == INNER // P - 1),
                        )
                    nc.vector.tensor_add(
                        ot[:, fh * FB:(fh + 1) * FB], po,
                        bias_sb[:, fh * FB:(fh + 1) * FB],
                    )
                nc.sync.dma_start(out[ic * P:(ic + 1) * P, :], ot)

    nc.finalize()
    return nc


def kernel(x, Wq, Wk, Wv, Wout, bout):
    global _NC, LAST_RESULTS
    x = np.asarray(x, dtype=np.float32)
    Wq = np.asarray(Wq, dtype=np.float32)
    Wk = np.asarray(Wk, dtype=np.float32)
    Wv = np.asarray(Wv, dtype=np.float32)
    Wout = np.asarray(Wout, dtype=np.float32)
    bout = np.asarray(bout, dtype=np.float32)

    if _NC is None:
        _NC = build_nc()

    xT = np.ascontiguousarray(x.reshape(BN, QDIM).T)
    bout2 = np.ascontiguousarray(bout.reshape(1, QDIM))
    in_maps = []
    for h in range(NCORES):
        in_maps.append({
            "xT": xT,
            "wq": np.ascontiguousarray(Wq[:, h * HD:(h + 1) * HD]),
            "wk": np.ascontiguousarray(Wk[:, h * HD:(h + 1) * HD]),
            "wv": np.ascontiguousarray(Wv[:, h * HD:(h + 1) * HD]),
            "wout": Wout,
            "bout": bout2,
        })

    res = run_bass_kernel_spmd(
        _NC, in_maps, core_ids=list(range(NCORES)),
        trace=bool(os.environ.get("BASS_TRACE")),
    )
    LAST_RESULTS = res
    full = np.concatenate(
        [res.results[i]["out"] for i in range(NCORES)], axis=0
    )
    return full.reshape(B, N, QDIM)
